# revision 8
# baseline (speedup 1.0000x reference)
"""Bass/Trainium2 kernel for nn_CrossAttention_33586644254982.

Math: the cross-attention has a single KV token, so softmax over the
key axis (size 1) is exactly 1.0 and the attention output equals V
broadcast over all N query positions. The full module therefore reduces to

    out[b, n, :] = (freq_token[b] @ Wv.T + bv) @ Wo.T + bo     (independent of n)

Q/K projections and spatial_tokens do not affect the output at all.
The two consecutive linear layers are then folded into one
(offline weight preprocessing, done host-side in float64):

    Wc = Wo @ Wv          [C, CFD]
    bc = Wo @ bv + bo     [C]
    out[b, n, :] = freq_token[b] @ Wc.T + bc

Strategy: data-parallel over B (16 batches -> 2 per core on 8 cores).
Per core: one matmul chain computes O = ft @ Wc.T streaming Wc as the
PE moving operand (fewest fp32 instruction pairs), with the folded
bias added in the PSUM->SBUF copy on DVE. GpSimd partition_broadcast
replicates each O row across 128 partitions (per N-half, so it starts
while the second half's matmuls finish), a DVE copy doubles it in the
free dim, and the 24 MiB output shard streams out as 16 large DMAs per
batch with 6 KiB descriptors (K_REP consecutive output rows per
partition) alternating between the SP and ACT HWDGE rings. Wc loads
split across both rings, first N-half first so mm h=0 starts early;
ft rides SWDGE. The store phase runs at the ~435 GB/s per-core HBM
cap (57.9 us floor for 24 MiB); everything before it is ~15 us
(7 us fixed framework preamble + load/compute/broadcast).
"""

import numpy as np

# Problem shapes (hardcoded per contract - kernel.py is self-contained).
B, N, C, CFD = 16, 4096, 768, 512
N_CORES = 8
BPC = B // N_CORES  # batches per core = 2
P = 128
KA = CFD // P       # k-chunks for the matmul = 4
K_REP = 2           # row-replicas materialized in SBUF per DMA burst
T = N // (K_REP * P)  # output DMAs per batch = 8

_CACHE = {}


def _build():
    from concourse import bacc, mybir
    from concourse.tile import TileContext

    f32 = mybir.dt.float32
    f16 = mybir.dt.float16
    nc = bacc.Bacc("TRN2", debug=False, num_devices=N_CORES)

    ftd = nc.dram_tensor("ftd", [P, KA, BPC], f16, kind="ExternalInput").ap()
    WcT = nc.dram_tensor("WcT", [CFD, C], f16, kind="ExternalInput").ap()
    bc2 = nc.dram_tensor("bc2", [BPC, C], f32, kind="ExternalInput").ap()
    out = nc.dram_tensor("out", [BPC, N, C], f32, kind="ExternalOutput").ap()

    with TileContext(nc) as tc:
        with (
            tc.tile_pool(name="consts", bufs=1) as consts,
            tc.tile_pool(name="weights", bufs=1) as weights,
            tc.tile_pool(name="small", bufs=1) as small,
            tc.tile_pool(name="repl", bufs=2) as replp,
            tc.tile_pool(name="ps_k", bufs=3, space="PSUM") as ps_k,
            tc.tile_pool(name="ps_warm", bufs=1, space="PSUM") as ps_warm,
        ):
            # ft (gates every matmul: stationary operand) and the tiny
            # bias go on SWDGE first - GpSimd is otherwise idle early and
            # the big HWDGE rings stay clear for the Wc chunks.
            ft_sb = consts.tile([P, KA, BPC], f16)
            nc.gpsimd.dma_start(out=ft_sb, in_=ftd)
            bc_sb = consts.tile([BPC, C], f32)
            nc.gpsimd.dma_start(out=bc_sb, in_=bc2)

            # Wc loads: full-row descriptors (half-row ones are
            # descriptor-rate-limited), chunk-per-ring across the two
            # HWDGE rings so a0+a1 land together.
            wc_sb = weights.tile([P, KA, C], f16)
            wc_view = WcT.rearrange("(a p) c -> a p c", p=P)
            NS1 = C // 2  # 384
            nc.sync.dma_start(out=wc_sb[:, 0, :], in_=wc_view[0])
            nc.scalar.dma_start(out=wc_sb[:, 1, :], in_=wc_view[1])
            nc.sync.dma_start(out=wc_sb[:, 2, :], in_=wc_view[2])
            nc.scalar.dma_start(out=wc_sb[:, 3, :], in_=wc_view[3])

            # Short PE warm-up on zeroed f16 scratch, sized to end right
            # as Wc's first chunks land, so the real chain runs warm.
            dum_l = consts.tile([P, P], f16)
            nc.vector.memset(dum_l, 0.0)
            dum_r = consts.tile([P, 512], f16)
            nc.vector.memset(dum_r, 0.0)
            ps_w = ps_warm.tile([P, 512], f32)
            for _ in range(3):
                nc.tensor.matmul(ps_w, dum_l, dum_r, start=True, stop=True)

            # O[b, j] = sum_k ft[b, k] Wc[j, k] + bc[j]
            # Wc streams as the moving operand (N=384) in f16: one PE pass
            # per logical matmul (fp32 needs two hi/lo passes and made the
            # PE chain the prologue bottleneck at ~7.8 us).
            # The two N-half accumulation chains interleave per k-chunk so
            # each Wc chunk arrival feeds both immediately.
            o_sb = small.tile([BPC, C], f32)
            pss = [
                ps_k.tile([BPC, NS1], f32, name=f"ps_h{h}") for h in range(2)
            ]
            for a in range(KA):
                for h in range(2):
                    nc.tensor.matmul(
                        pss[h],
                        ft_sb[:, a, :],
                        wc_sb[:, a, h * NS1 : (h + 1) * NS1],
                        start=(a == 0),
                        stop=(a == KA - 1),
                    )
            for h in range(2):
                sl = slice(h * NS1, (h + 1) * NS1)
                nc.vector.tensor_add(o_sb[:, sl], pss[h], bc_sb[:, sl])

            # O rows at partition 0: b=0 aliases o_sb row 0; b=1 moves to
            # partition 0 via a tiny SBUF->SBUF DMA (DMAs have no
            # partition-base restriction, unlike compute engines). The DMA
            # is emitted inside the b-loop AFTER pb(b=0) so the in-order
            # GpSimd stream broadcasts b=0 as soon as o_sb lands.
            orow1 = small.tile([1, C], f32)
            orow = [o_sb[0:1, :], orow1]

            # Broadcast O rows across partitions (per N-half, so the h=0
            # broadcast overlaps the h=1 matmul tail), replicate K_REP
            # times in the free dim, and stream the output shard with
            # 6 KiB descriptors (q=K_REP consecutive output rows per
            # partition).
            outv = out.rearrange("b (t p q) c -> b t p (q c)", p=P, q=K_REP)
            engines = [nc.sync, nc.scalar]
            di = 0
            for b in range(BPC):
                r4 = replp.tile([P, K_REP, C], f32)
                for h in range(2):
                    sl = slice(h * NS1, (h + 1) * NS1)
                    nc.gpsimd.partition_broadcast(r4[:, 0, sl], orow[b][:, sl])
                if b + 1 < BPC:
                    nc.gpsimd.dma_start(out=orow1, in_=o_sb[b + 1 : b + 2, :])
                for rep in range(1, K_REP):
                    nc.vector.tensor_copy(r4[:, rep, :], r4[:, 0, :])
                r4_flat = r4.rearrange("p r c -> p (r c)")
                for t in range(T):
                    engines[di % 2].dma_start(out=outv[b, t], in_=r4_flat)
                    di += 1

    nc.compile()
    return nc


def _get_nc():
    if "nc" not in _CACHE:
        _CACHE["nc"] = _build()
    return _CACHE["nc"]


def _install_ntff_hook():
    """Provide antenv.axon_hooks if the image lacks it (profiling only)."""
    import sys
    import types

    try:
        from antenv.axon_hooks import get_axon_ntff_profile_hook  # noqa: F401

        return
    except ImportError:
        pass
    try:
        import antenv
        from trn_agent_boot.trn_boot import _ntff_profile_via_ctypes

        hook = _ntff_profile_via_ctypes("/opt/axon/libaxon_pjrt.so")
        mod = types.ModuleType("antenv.axon_hooks")
        mod.get_axon_ntff_profile_hook = lambda: hook
        mod.set_axon_ntff_profile_hook = lambda h: None
        sys.modules["antenv.axon_hooks"] = mod
        antenv.axon_hooks = mod
    except Exception as e:  # pragma: no cover - profiling is best-effort
        print(f"ntff hook install failed ({e}); tracing disabled", file=sys.stderr)


def _run(inputs, trace=False):
    from concourse import bass_utils

    if trace:
        _install_ntff_hook()
        # Zero-egress container: skip the artifact upload, keep files local.
        bass_utils.upload_artifacts = lambda tmpdir: tmpdir

    nc = _get_nc()
    ft = np.asarray(inputs["freq_token"], np.float32)
    # Fold the two linear layers (weight preprocessing, float64 for
    # accuracy): out_row = ft @ (Wo @ Wv).T + (Wo @ bv + bo).
    Wv64 = np.asarray(inputs["Wv"], np.float64)
    Wo64 = np.asarray(inputs["Wo"], np.float64)
    bv64 = np.asarray(inputs["bv"], np.float64)
    bo64 = np.asarray(inputs["bo"], np.float64)
    WcT = np.ascontiguousarray((Wv64.T @ Wo64.T).astype(np.float16))  # [CFD, C]
    bc = (Wo64 @ bv64 + bo64).astype(np.float32)  # [C]
    # Bias rows duplicated per batch so DVE tensor_add partitions line up.
    bc2 = np.ascontiguousarray(np.broadcast_to(bc, (BPC, C)))

    in_maps = []
    for i in range(N_CORES):
        ft_loc = ft[BPC * i : BPC * (i + 1)]  # [BPC, CFD]
        # ftd[p, a, b] = ft_loc[b, a*128 + p]
        ftd = np.ascontiguousarray(
            ft_loc.T.reshape(KA, P, BPC).transpose(1, 0, 2)
        ).astype(np.float16)
        in_maps.append({"ftd": ftd, "WcT": WcT, "bc2": bc2})
    res = bass_utils.run_bass_kernel_spmd(
        nc, in_maps, core_ids=list(range(N_CORES)), trace=trace
    )
    out = np.concatenate([m["out"] for m in res.results], axis=0)
    return out, res


def kernel(**inputs):
    out, _ = _run(inputs, trace=False)
    return out


# revision 10
# speedup vs baseline: 1.0007x; 1.0007x over previous
"""Bass/Trainium2 kernel for nn_CrossAttention_33586644254982.

Math: the cross-attention has a single KV token, so softmax over the
key axis (size 1) is exactly 1.0 and the attention output equals V
broadcast over all N query positions. The full module therefore reduces to

    out[b, n, :] = (freq_token[b] @ Wv.T + bv) @ Wo.T + bo     (independent of n)

Q/K projections and spatial_tokens do not affect the output at all.
The two consecutive linear layers are folded into one (offline weight
preprocessing, done host-side in float64):

    Wc = Wo @ Wv          [C, CFD]
    bc = Wo @ bv + bo     [C]
    out[b, n, :] = freq_token[b] @ Wc.T + bc

Strategy: data-parallel over B (16 batches -> 2 per core on 8 cores).
Per core, the critical path to the first output store is:
  - ft (16 B/partition) split across both HWDGE rings first, then the
    four Wc k-chunks, each partition-split across the rings (full-row
    descriptors; half-row ones are descriptor-rate-limited).
  - 8 f16 matmul passes (2 PSUM-bank halves x 4 k-chunks, interleaved
    so each chunk arrival feeds both chains; f16 = 1 pass/matmul where
    fp32 needs 2) with the folded bias added in the PSUM->SBUF move.
  - Each O row is broadcast across 128 partitions by a rank-1 PE
    matmul (ones[1,128] stationary) - the GpSimd partition_broadcast
    used before has ~2 us semaphore-wake latency - then copied
    PSUM->SBUF by DVE.
  - The 24 MiB output shard streams out as 32 DMAs per batch with
    3 KiB descriptors alternating between the SP and ACT HWDGE rings.
The store phase runs at the per-core HBM write bandwidth (~415 GB/s
uncontended, ~345 GB/s when all 8 cores' stores overlap, which they
do here), so ~61-73 us of the total is the unavoidable output write;
everything before the first store is ~15 us (7 us fixed framework
preamble + load/compute/broadcast).
"""

import numpy as np

# Problem shapes (hardcoded per contract - kernel.py is self-contained).
B, N, C, CFD = 16, 4096, 768, 512
N_CORES = 8
BPC = B // N_CORES  # batches per core = 2
P = 128
KA = CFD // P       # k-chunks for the matmul = 4
T = N // P          # output DMAs per batch = 32

_CACHE = {}


def _build():
    from concourse import bacc, mybir
    from concourse.tile import TileContext

    f32 = mybir.dt.float32
    f16 = mybir.dt.float16
    nc = bacc.Bacc("TRN2", debug=False, num_devices=N_CORES)

    ftd = nc.dram_tensor("ftd", [P, KA, BPC], f16, kind="ExternalInput").ap()
    WcT = nc.dram_tensor("WcT", [CFD, C], f16, kind="ExternalInput").ap()
    bc2 = nc.dram_tensor("bc2", [BPC, C], f32, kind="ExternalInput").ap()
    out = nc.dram_tensor("out", [BPC, N, C], f32, kind="ExternalOutput").ap()

    with TileContext(nc) as tc:
        with (
            tc.tile_pool(name="consts", bufs=1) as consts,
            tc.tile_pool(name="weights", bufs=1) as weights,
            tc.tile_pool(name="small", bufs=1) as small,
            tc.tile_pool(name="repl", bufs=2) as replp,
            tc.tile_pool(name="ps_k", bufs=1, space="PSUM") as ps_k,
            tc.tile_pool(name="ps_b", bufs=1, space="PSUM") as ps_b,
            tc.tile_pool(name="ps_warm", bufs=1, space="PSUM") as ps_warm,
        ):
            # ft gates every matmul (stationary operand): load it first,
            # partition-split across both HWDGE rings. The tiny bias rides
            # along. Then the Wc chunks, each partition-split as well so
            # chunk a lands before chunk a+1 starts.
            ft_sb = consts.tile([P, KA, BPC], f16)
            nc.sync.dma_start(out=ft_sb[0:64], in_=ftd[0:64])
            nc.scalar.dma_start(out=ft_sb[64:128], in_=ftd[64:128])
            bc_sb = consts.tile([BPC, C], f32)
            nc.sync.dma_start(out=bc_sb, in_=bc2)

            wc_sb = weights.tile([P, KA, C], f16)
            wc_view = WcT.rearrange("(a p) c -> a p c", p=P)
            NS1 = C // 2  # 384
            for a in range(KA):
                nc.sync.dma_start(out=wc_sb[0:64, a, :], in_=wc_view[a][0:64])
                nc.scalar.dma_start(
                    out=wc_sb[64:128, a, :], in_=wc_view[a][64:128]
                )

            # Constants for the PE ops; short PE warm-up on zeroed f16
            # scratch while the first loads land.
            ones = consts.tile([1, P], f32)
            nc.vector.memset(ones, 1.0)
            dum_l = consts.tile([P, P], f16)
            nc.vector.memset(dum_l, 0.0)
            dum_r = consts.tile([P, 512], f16)
            nc.vector.memset(dum_r, 0.0)
            ps_w = ps_warm.tile([P, 512], f32)
            for _ in range(2):
                nc.tensor.matmul(ps_w, dum_l, dum_r, start=True, stop=True)

            # O[b, j] = sum_k ft[b, k] Wc[j, k] + bc[j]
            # Wc streams as the moving operand (N=384) in f16: one PE pass
            # per logical matmul. The two N-half accumulation chains
            # interleave per k-chunk so each chunk arrival feeds both.
            o_sb = small.tile([BPC, C], f32)
            pss = [
                ps_k.tile([BPC, NS1], f32, name=f"ps_h{h}") for h in range(2)
            ]
            for a in range(KA):
                for h in range(2):
                    nc.tensor.matmul(
                        pss[h],
                        ft_sb[:, a, :],
                        wc_sb[:, a, h * NS1 : (h + 1) * NS1],
                        start=(a == 0),
                        stop=(a == KA - 1),
                    )
            for h in range(2):
                sl = slice(h * NS1, (h + 1) * NS1)
                nc.vector.tensor_add(o_sb[:, sl], pss[h], bc_sb[:, sl])

            # O rows at partition 0: b=0 aliases o_sb row 0; b=1 moves to
            # partition 0 via a tiny SBUF->SBUF DMA (DMAs have no
            # partition-base restriction, unlike compute engines). GpSimd
            # only does this one transfer; it is off the critical path
            # (b=1's stores queue behind b=0's 32 anyway).
            orow1 = small.tile([1, C], f32)
            nc.gpsimd.dma_start(out=orow1, in_=o_sb[1:2, :])
            orow = [o_sb[0:1, :], orow1]

            # Broadcast O rows across partitions with a rank-1 PE matmul
            # per N-half (PSUM-bank sized), copy PSUM->SBUF on DVE, and
            # stream the output shard with 3 KiB descriptors (one output
            # row per partition per DMA).
            outv = out.rearrange("b (t p) c -> b t p c", p=P)
            engines = [nc.sync, nc.scalar]
            di = 0
            for b in range(BPC):
                r4 = replp.tile([P, C], f32)
                for h in range(2):
                    sl = slice(h * NS1, (h + 1) * NS1)
                    ps2 = ps_b.tile([P, NS1], f32, name=f"ps2_b{b}h{h}")
                    nc.tensor.matmul(
                        ps2, ones, orow[b][:, sl], start=True, stop=True
                    )
                    nc.vector.tensor_copy(r4[:, sl], ps2)
                for t in range(T):
                    engines[di % 2].dma_start(out=outv[b, t], in_=r4)
                    di += 1

    nc.compile()
    return nc


def _get_nc():
    if "nc" not in _CACHE:
        _CACHE["nc"] = _build()
    return _CACHE["nc"]


def _install_ntff_hook():
    """Provide antenv.axon_hooks if the image lacks it (profiling only)."""
    import sys
    import types

    try:
        from antenv.axon_hooks import get_axon_ntff_profile_hook  # noqa: F401

        return
    except ImportError:
        pass
    try:
        import antenv
        from trn_agent_boot.trn_boot import _ntff_profile_via_ctypes

        hook = _ntff_profile_via_ctypes("/opt/axon/libaxon_pjrt.so")
        mod = types.ModuleType("antenv.axon_hooks")
        mod.get_axon_ntff_profile_hook = lambda: hook
        mod.set_axon_ntff_profile_hook = lambda h: None
        sys.modules["antenv.axon_hooks"] = mod
        antenv.axon_hooks = mod
    except Exception as e:  # pragma: no cover - profiling is best-effort
        print(f"ntff hook install failed ({e}); tracing disabled", file=sys.stderr)


def _run(inputs, trace=False):
    from concourse import bass_utils

    if trace:
        _install_ntff_hook()
        # Zero-egress container: skip the artifact upload, keep files local.
        bass_utils.upload_artifacts = lambda tmpdir: tmpdir

    nc = _get_nc()
    ft = np.asarray(inputs["freq_token"], np.float32)
    # Fold the two linear layers (weight preprocessing, float64 for
    # accuracy): out_row = ft @ (Wo @ Wv).T + (Wo @ bv + bo).
    Wv64 = np.asarray(inputs["Wv"], np.float64)
    Wo64 = np.asarray(inputs["Wo"], np.float64)
    bv64 = np.asarray(inputs["bv"], np.float64)
    bo64 = np.asarray(inputs["bo"], np.float64)
    WcT = np.ascontiguousarray((Wv64.T @ Wo64.T).astype(np.float16))  # [CFD, C]
    bc = (Wo64 @ bv64 + bo64).astype(np.float32)  # [C]
    # Bias rows duplicated per batch so DVE tensor_add partitions line up.
    bc2 = np.ascontiguousarray(np.broadcast_to(bc, (BPC, C)))

    in_maps = []
    for i in range(N_CORES):
        ft_loc = ft[BPC * i : BPC * (i + 1)]  # [BPC, CFD]
        # ftd[p, a, b] = ft_loc[b, a*128 + p]
        ftd = np.ascontiguousarray(
            ft_loc.T.reshape(KA, P, BPC).transpose(1, 0, 2)
        ).astype(np.float16)
        in_maps.append({"ftd": ftd, "WcT": WcT, "bc2": bc2})
    res = bass_utils.run_bass_kernel_spmd(
        nc, in_maps, core_ids=list(range(N_CORES)), trace=trace
    )
    out = np.concatenate([m["out"] for m in res.results], axis=0)
    return out, res


def kernel(**inputs):
    out, _ = _run(inputs, trace=False)
    return out


# revision 11
# speedup vs baseline: 1.1409x; 1.1401x over previous
"""Bass/Trainium2 kernel for nn_CrossAttention_33586644254982.

Math: the cross-attention has a single KV token, so softmax over the
key axis (size 1) is exactly 1.0 and the attention output equals V
broadcast over all N query positions. The full module therefore reduces to

    out[b, n, :] = (freq_token[b] @ Wv.T + bv) @ Wo.T + bo     (independent of n)

Q/K projections and spatial_tokens do not affect the output at all.
The two consecutive linear layers are folded into one (offline weight
preprocessing, done host-side in float64):

    Wc = Wo @ Wv          [C, CFD]
    bc = Wo @ bv + bo     [C]
    out[b, n, :] = freq_token[b] @ Wc.T + bc

Strategy: data-parallel over B (16 batches -> 2 per core on 8 cores).
Per core, the kernel computes the matmul directly in BROADCAST form:
the stationary operand is ft_b's k-chunk replicated across all 128
M-columns (prepared host-side), so each PSUM result tile [128, 384]
holds the O row already broadcast across partitions - no separate
row-extract / partition-broadcast / copy chain, and GpSimd is not used
at all. The DVE tensor_add that moves PSUM->SBUF adds the
host-broadcast folded bias. f16 operands: one PE pass per matmul
(fp32 needs two) - the PE runs at its cold 1.2 GHz clock this early in
the kernel, so pass count dominates the compute phase.

Loads: weights are packed host-side so each partition's rows for two
k-chunks are contiguous (3 KiB descriptors instead of 1.5 KiB - the
HWDGE rings process ~1 descriptor per 14 ns per ring, which is the
load bottleneck). Everything is partition-split across the SP and ACT
rings. The 24 MiB output shard then streams out as 32 DMAs per batch
with 3 KiB descriptors (one output row per partition) alternating
between the rings.

The store phase runs at the per-core HBM write bandwidth (~415 GB/s
uncontended, ~345 GB/s when all 8 cores' stores overlap, which they
do here), so ~61-73 us of the total is the unavoidable output write;
everything before the first store is ~15 us (7 us fixed framework
preamble + load/compute).
"""

import numpy as np

# Problem shapes (hardcoded per contract - kernel.py is self-contained).
B, N, C, CFD = 16, 4096, 768, 512
N_CORES = 8
BPC = B // N_CORES  # batches per core = 2
P = 128
KA = CFD // P       # k-chunks for the matmul = 4
PAIRS = KA // 2     # k-chunk pairs packed per load descriptor = 2
T = N // P          # output DMAs per batch = 32
H = 64              # partition-split point for ring load balancing

_CACHE = {}


def _build():
    from concourse import bacc, mybir
    from concourse.tile import TileContext

    f32 = mybir.dt.float32
    f16 = mybir.dt.float16
    nc = bacc.Bacc("TRN2", debug=False, num_devices=N_CORES)

    ftb = nc.dram_tensor("ftb", [P, KA, BPC, P], f16, kind="ExternalInput").ap()
    wcp = nc.dram_tensor("wcp", [P, PAIRS, 2, C], f16, kind="ExternalInput").ap()
    bcb = nc.dram_tensor("bcb", [P, C], f32, kind="ExternalInput").ap()
    out = nc.dram_tensor("out", [BPC, N, C], f32, kind="ExternalOutput").ap()

    with TileContext(nc) as tc:
        with (
            tc.tile_pool(name="consts", bufs=1) as consts,
            tc.tile_pool(name="weights", bufs=1) as weights,
            tc.tile_pool(name="repl", bufs=2) as replp,
            tc.tile_pool(name="ps_b", bufs=1, space="PSUM") as ps_b,
            tc.tile_pool(name="ps_warm", bufs=1, space="PSUM") as ps_warm,
        ):
            # Loads, everything partition-split across the two HWDGE
            # rings: ft-broadcast first (gates every matmul), then the
            # two Wc chunk-pairs, then the broadcast bias (needed last).
            ft_sb = consts.tile([P, KA, BPC, P], f16)
            wc_sb = weights.tile([P, PAIRS, 2, C], f16)
            bc_sb = consts.tile([P, C], f32)
            for eng, sl in ((nc.sync, slice(0, H)), (nc.scalar, slice(H, P))):
                eng.dma_start(out=ft_sb[sl], in_=ftb[sl])
                eng.dma_start(out=wc_sb[sl, 0], in_=wcp[sl, 0])
                eng.dma_start(out=wc_sb[sl, 1], in_=wcp[sl, 1])
                eng.dma_start(out=bc_sb[sl], in_=bcb[sl])

            # Short PE warm-up on zeroed f16 scratch while loads land.
            dum_l = consts.tile([P, P], f16)
            nc.vector.memset(dum_l, 0.0)
            dum_r = consts.tile([P, 512], f16)
            nc.vector.memset(dum_r, 0.0)
            ps_w = ps_warm.tile([P, 512], f32)
            for _ in range(2):
                nc.tensor.matmul(ps_w, dum_l, dum_r, start=True, stop=True)

            # r4[p, j] = sum_k ftb[k, *, b, p] Wc[j, k] + bc[j]  (same for
            # every p). Two PSUM-bank halves (N=384) x 4 k-chunks per
            # batch; batch 0 fully first so its stores start earliest.
            NS1 = C // 2  # 384
            outv = out.rearrange("b (t p) c -> b t p c", p=P)
            engines = [nc.sync, nc.scalar]
            di = 0
            for b in range(BPC):
                pss = [
                    ps_b.tile([P, NS1], f32, name=f"ps_b{b}h{h}")
                    for h in range(2)
                ]
                for a in range(KA):
                    for h in range(2):
                        nc.tensor.matmul(
                            pss[h],
                            ft_sb[:, a, b, :],
                            wc_sb[:, a // 2, a % 2, h * NS1 : (h + 1) * NS1],
                            start=(a == 0),
                            stop=(a == KA - 1),
                        )
                r4 = replp.tile([P, C], f32)
                for h in range(2):
                    sl = slice(h * NS1, (h + 1) * NS1)
                    nc.vector.tensor_add(r4[:, sl], pss[h], bc_sb[:, sl])
                for t in range(T):
                    engines[di % 2].dma_start(out=outv[b, t], in_=r4)
                    di += 1

    nc.compile()
    return nc


def _get_nc():
    if "nc" not in _CACHE:
        _CACHE["nc"] = _build()
    return _CACHE["nc"]


def _install_ntff_hook():
    """Provide antenv.axon_hooks if the image lacks it (profiling only)."""
    import sys
    import types

    try:
        from antenv.axon_hooks import get_axon_ntff_profile_hook  # noqa: F401

        return
    except ImportError:
        pass
    try:
        import antenv
        from trn_agent_boot.trn_boot import _ntff_profile_via_ctypes

        hook = _ntff_profile_via_ctypes("/opt/axon/libaxon_pjrt.so")
        mod = types.ModuleType("antenv.axon_hooks")
        mod.get_axon_ntff_profile_hook = lambda: hook
        mod.set_axon_ntff_profile_hook = lambda h: None
        sys.modules["antenv.axon_hooks"] = mod
        antenv.axon_hooks = mod
    except Exception as e:  # pragma: no cover - profiling is best-effort
        print(f"ntff hook install failed ({e}); tracing disabled", file=sys.stderr)


def _run(inputs, trace=False):
    from concourse import bass_utils

    if trace:
        _install_ntff_hook()
        # Zero-egress container: skip the artifact upload, keep files local.
        bass_utils.upload_artifacts = lambda tmpdir: tmpdir

    nc = _get_nc()
    ft = np.asarray(inputs["freq_token"], np.float32)
    # Fold the two linear layers (weight preprocessing, float64 for
    # accuracy): out_row = ft @ (Wo @ Wv).T + (Wo @ bv + bo).
    Wv64 = np.asarray(inputs["Wv"], np.float64)
    Wo64 = np.asarray(inputs["Wo"], np.float64)
    bv64 = np.asarray(inputs["bv"], np.float64)
    bo64 = np.asarray(inputs["bo"], np.float64)
    WcT = (Wv64.T @ Wo64.T).astype(np.float16)  # [CFD, C]
    bc = (Wo64 @ bv64 + bo64).astype(np.float32)  # [C]

    # wcp[p, q, j, c] = WcT[(2q+j)*128 + p, c]: each partition's rows for
    # a chunk-pair are contiguous -> 3 KiB load descriptors.
    wcp = np.ascontiguousarray(
        WcT.reshape(PAIRS, 2, P, C).transpose(2, 0, 1, 3)
    )
    bcb = np.ascontiguousarray(np.broadcast_to(bc, (P, C)))

    in_maps = []
    for i in range(N_CORES):
        ft_loc = ft[BPC * i : BPC * (i + 1)]  # [BPC, CFD]
        # ftb[k, a, b, m] = ft_loc[b, a*128 + k] for every m (stationary
        # operand replicated across the 128 M-columns = broadcast form).
        ftb = np.ascontiguousarray(
            np.broadcast_to(
                ft_loc.T.reshape(KA, P, BPC).transpose(1, 0, 2)[:, :, :, None],
                (P, KA, BPC, P),
            ).astype(np.float16)
        )
        in_maps.append({"ftb": ftb, "wcp": wcp, "bcb": bcb})
    res = bass_utils.run_bass_kernel_spmd(
        nc, in_maps, core_ids=list(range(N_CORES)), trace=trace
    )
    out = np.concatenate([m["out"] for m in res.results], axis=0)
    return out, res


def kernel(**inputs):
    out, _ = _run(inputs, trace=False)
    return out


# revision 15
# speedup vs baseline: 1.5961x; 1.3989x over previous
"""Bass/Trainium2 kernel for nn_CrossAttention_33586644254982.

Math: the cross-attention has a single KV token, so softmax over the
key axis (size 1) is exactly 1.0 and the attention output equals V
broadcast over all N query positions. The full module therefore reduces to

    out[b, n, :] = (freq_token[b] @ Wv.T + bv) @ Wo.T + bo     (independent of n)

Q/K projections and spatial_tokens do not affect the output at all.
The two consecutive linear layers are folded into one (offline weight
preprocessing, done host-side in float64):

    Wc = Wo @ Wv          [C, CFD]
    bc = Wo @ bv + bo     [C]
    out[b, n, :] = freq_token[b] @ Wc.T + bc

Strategy: data-parallel over B (16 batches -> 2 per core on 8 cores).
Per core, the kernel computes the matmul directly in BROADCAST form:
the stationary operand is ft_b's k-chunk replicated across all 128
M-columns (prepared host-side), so each PSUM result tile [128, 384]
holds the O row already broadcast across partitions - no separate
row-extract / partition-broadcast / copy chain, and GpSimd is not used
at all. The DVE tensor_add that moves PSUM->SBUF adds the
host-broadcast folded bias. f16 operands: one PE pass per matmul
(fp32 needs two) - the PE runs at its cold 1.2 GHz clock this early in
the kernel, so pass count dominates the compute phase.

Loads: weights are packed host-side so each partition's rows for two
k-chunks are contiguous (3 KiB descriptors instead of 1.5 KiB - the
HWDGE rings process ~1 descriptor per 14 ns per ring, which is the
load bottleneck). Everything is partition-split across the SP and ACT
rings. The 24 MiB output shard then streams out as 32 DMAs per batch
with 3 KiB descriptors (one output row per partition) alternating
between the rings.

The store phase runs at the per-core HBM write bandwidth (~415 GB/s
uncontended, ~345 GB/s when all 8 cores' stores overlap, which they
do here), so ~61-73 us of the total is the unavoidable output write;
everything before the first store is ~15 us (7 us fixed framework
preamble + load/compute).
"""

import numpy as np

# Problem shapes (hardcoded per contract - kernel.py is self-contained).
B, N, C, CFD = 16, 4096, 768, 512
N_CORES = 8
BPC = B // N_CORES  # batches per core = 2
P = 128
KA = CFD // P       # k-chunks for the matmul = 4
PAIRS = KA // 2     # k-chunk pairs packed per load descriptor = 2
K_REP = 2           # output rows per partition per store descriptor
T = N // (K_REP * P)  # output DMAs per batch = 16
H = 64              # partition-split point for ring load balancing

_CACHE = {}


def _build():
    from concourse import bacc, mybir
    from concourse.tile import TileContext

    f32 = mybir.dt.float32
    f16 = mybir.dt.float16
    nc = bacc.Bacc("TRN2", debug=False, num_devices=N_CORES)

    ftb = nc.dram_tensor("ftb", [P, KA, BPC, P], f16, kind="ExternalInput").ap()
    wcp = nc.dram_tensor("wcp", [P, PAIRS, 2, C], f16, kind="ExternalInput").ap()
    bcb = nc.dram_tensor("bcb", [P, C], f32, kind="ExternalInput").ap()
    # Output in f16: halves the dominant HBM store traffic (12 MiB/core
    # instead of 24); the host upcasts to f32 during unshard. Rounding
    # adds ~2.4e-4 relative error against a 2e-2 budget.
    out = nc.dram_tensor("out", [BPC, N, C], f16, kind="ExternalOutput").ap()

    with TileContext(nc) as tc:
        with (
            tc.tile_pool(name="consts", bufs=1) as consts,
            tc.tile_pool(name="weights", bufs=1) as weights,
            tc.tile_pool(name="repl", bufs=2) as replp,
            tc.tile_pool(name="ps_b", bufs=1, space="PSUM") as ps_b,
            tc.tile_pool(name="ps_warm", bufs=1, space="PSUM") as ps_warm,
        ):
            # Loads, everything partition-split across the two HWDGE
            # rings: ft-broadcast first (gates every matmul), then the
            # two Wc chunk-pairs, then the broadcast bias (needed last).
            ft_sb = consts.tile([P, KA, BPC, P], f16)
            wc_sb = weights.tile([P, PAIRS, 2, C], f16)
            bc_sb = consts.tile([P, C], f32)
            for eng, sl in ((nc.sync, slice(0, H)), (nc.scalar, slice(H, P))):
                eng.dma_start(out=ft_sb[sl], in_=ftb[sl])
                eng.dma_start(out=wc_sb[sl, 0], in_=wcp[sl, 0])
                eng.dma_start(out=wc_sb[sl, 1], in_=wcp[sl, 1])
                eng.dma_start(out=bc_sb[sl], in_=bcb[sl])

            # Short PE warm-up on zeroed f16 scratch while loads land.
            dum_l = consts.tile([P, P], f16)
            nc.vector.memset(dum_l, 0.0)
            dum_r = consts.tile([P, 512], f16)
            nc.vector.memset(dum_r, 0.0)
            ps_w = ps_warm.tile([P, 512], f32)
            for _ in range(2):
                nc.tensor.matmul(ps_w, dum_l, dum_r, start=True, stop=True)

            # r4[p, j] = sum_k ftb[k, *, b, p] Wc[j, k] + bc[j]  (same for
            # every p). Two PSUM-bank halves (N=384) x 4 k-chunks per
            # batch; batch 0 fully first so its stores start earliest.
            NS1 = C // 2  # 384
            outv = out.rearrange("b (t p q) c -> b t p (q c)", p=P, q=K_REP)
            engines = [nc.sync, nc.scalar]
            di = 0
            for b in range(BPC):
                pss = [
                    ps_b.tile([P, NS1], f32, name=f"ps_b{b}h{h}")
                    for h in range(2)
                ]
                for a in range(KA):
                    for h in range(2):
                        nc.tensor.matmul(
                            pss[h],
                            ft_sb[:, a, b, :],
                            wc_sb[:, a // 2, a % 2, h * NS1 : (h + 1) * NS1],
                            start=(a == 0),
                            stop=(a == KA - 1),
                        )
                if b == 0:
                    # Two trivial PE ops so the b=0 stop-passes retire
                    # promptly (PSUM-ready semaphores post ~2 instructions
                    # behind issue; b=1's 527 ns passes would add ~1 us).
                    for _ in range(2):
                        nc.tensor.matmul(
                            ps_w[:, 0:1],
                            dum_l,
                            dum_r[:, 0:1],
                            start=True,
                            stop=True,
                        )
                r4 = replp.tile([P, K_REP, C], f16)
                for h in range(2):
                    sl = slice(h * NS1, (h + 1) * NS1)
                    nc.vector.tensor_add(r4[:, 0, sl], pss[h], bc_sb[:, sl])
                nc.vector.tensor_copy(r4[:, 1, :], r4[:, 0, :])
                r4_flat = r4.rearrange("p r c -> p (r c)")
                for t in range(T):
                    engines[di % 2].dma_start(out=outv[b, t], in_=r4_flat)
                    di += 1

    nc.compile()
    return nc


def _get_nc():
    if "nc" not in _CACHE:
        _CACHE["nc"] = _build()
    return _CACHE["nc"]


def _install_ntff_hook():
    """Provide antenv.axon_hooks if the image lacks it (profiling only)."""
    import sys
    import types

    try:
        from antenv.axon_hooks import get_axon_ntff_profile_hook  # noqa: F401

        return
    except ImportError:
        pass
    try:
        import antenv
        from trn_agent_boot.trn_boot import _ntff_profile_via_ctypes

        hook = _ntff_profile_via_ctypes("/opt/axon/libaxon_pjrt.so")
        mod = types.ModuleType("antenv.axon_hooks")
        mod.get_axon_ntff_profile_hook = lambda: hook
        mod.set_axon_ntff_profile_hook = lambda h: None
        sys.modules["antenv.axon_hooks"] = mod
        antenv.axon_hooks = mod
    except Exception as e:  # pragma: no cover - profiling is best-effort
        print(f"ntff hook install failed ({e}); tracing disabled", file=sys.stderr)


def _run(inputs, trace=False):
    from concourse import bass_utils

    if trace:
        _install_ntff_hook()
        # Zero-egress container: skip the artifact upload, keep files local.
        bass_utils.upload_artifacts = lambda tmpdir: tmpdir

    nc = _get_nc()
    ft = np.asarray(inputs["freq_token"], np.float32)
    # Fold the two linear layers (weight preprocessing, float64 for
    # accuracy): out_row = ft @ (Wo @ Wv).T + (Wo @ bv + bo).
    Wv64 = np.asarray(inputs["Wv"], np.float64)
    Wo64 = np.asarray(inputs["Wo"], np.float64)
    bv64 = np.asarray(inputs["bv"], np.float64)
    bo64 = np.asarray(inputs["bo"], np.float64)
    WcT = (Wv64.T @ Wo64.T).astype(np.float16)  # [CFD, C]
    bc = (Wo64 @ bv64 + bo64).astype(np.float32)  # [C]

    # wcp[p, q, j, c] = WcT[(2q+j)*128 + p, c]: each partition's rows for
    # a chunk-pair are contiguous -> 3 KiB load descriptors.
    wcp = np.ascontiguousarray(
        WcT.reshape(PAIRS, 2, P, C).transpose(2, 0, 1, 3)
    )
    bcb = np.ascontiguousarray(np.broadcast_to(bc, (P, C)))

    in_maps = []
    for i in range(N_CORES):
        ft_loc = ft[BPC * i : BPC * (i + 1)]  # [BPC, CFD]
        # ftb[k, a, b, m] = ft_loc[b, a*128 + k] for every m (stationary
        # operand replicated across the 128 M-columns = broadcast form).
        ftb = np.ascontiguousarray(
            np.broadcast_to(
                ft_loc.T.reshape(KA, P, BPC).transpose(1, 0, 2)[:, :, :, None],
                (P, KA, BPC, P),
            ).astype(np.float16)
        )
        in_maps.append({"ftb": ftb, "wcp": wcp, "bcb": bcb})
    res = bass_utils.run_bass_kernel_spmd(
        nc, in_maps, core_ids=list(range(N_CORES)), trace=trace
    )
    out = np.concatenate(
        [m["out"].astype(np.float32) for m in res.results], axis=0
    )
    return out, res


def kernel(**inputs):
    out, _ = _run(inputs, trace=False)
    return out


# revision 17
# speedup vs baseline: 1.7982x; 1.1267x over previous
"""Bass/Trainium2 kernel for nn_CrossAttention_33586644254982.

Math: the cross-attention has a single KV token, so softmax over the
key axis (size 1) is exactly 1.0 and the attention output equals V
broadcast over all N query positions. The full module therefore reduces to

    out[b, n, :] = (freq_token[b] @ Wv.T + bv) @ Wo.T + bo     (independent of n)

Q/K projections and spatial_tokens do not affect the output at all.
The two consecutive linear layers are folded into one (offline weight
preprocessing, done host-side in float64):

    Wc = Wo @ Wv          [C, CFD]
    bc = Wo @ bv + bo     [C]
    out[b, n, :] = freq_token[b] @ Wc.T + bc

Strategy: data-parallel over B (16 batches -> 2 per core on 8 cores).
Per core, the kernel computes the matmul directly in BROADCAST form:
the stationary operand is ft_b's k-chunk replicated across all 128
M-columns (prepared host-side), so each PSUM result tile [128, 384]
holds the O row already broadcast across partitions - no separate
row-extract / partition-broadcast / copy chain, and GpSimd is not used
at all. The DVE tensor_add that moves PSUM->SBUF adds the
host-broadcast folded bias. f16 operands: one PE pass per matmul
(fp32 needs two) - the PE runs at its cold 1.2 GHz clock this early in
the kernel, so pass count dominates the compute phase.

Loads: weights are packed host-side so each partition's rows for two
k-chunks are contiguous (3 KiB descriptors instead of 1.5 KiB - the
HWDGE rings process ~1 descriptor per 14 ns per ring, which is the
load bottleneck). Everything is partition-split across the SP and ACT
rings. The 24 MiB output shard then streams out as 32 DMAs per batch
with 3 KiB descriptors (one output row per partition) alternating
between the rings.

The store phase runs at the per-core HBM write bandwidth (~415 GB/s
uncontended, ~345 GB/s when all 8 cores' stores overlap, which they
do here), so ~61-73 us of the total is the unavoidable output write;
everything before the first store is ~15 us (7 us fixed framework
preamble + load/compute).
"""

import numpy as np

# Problem shapes (hardcoded per contract - kernel.py is self-contained).
B, N, C, CFD = 16, 4096, 768, 512
N_CORES = 8
BPC = B // N_CORES  # batches per core = 2
P = 128
KA = CFD // P       # k-chunks for the matmul = 4
PAIRS = KA // 2     # k-chunk pairs packed per load descriptor = 2
K_REP = 2           # output rows per partition per store descriptor
T = N // (K_REP * P)  # output DMAs per batch = 16
H = 64              # partition-split point for ring load balancing

_CACHE = {}


def _build():
    from concourse import bacc, mybir
    from concourse.tile import TileContext

    f32 = mybir.dt.float32
    f16 = mybir.dt.float16
    nc = bacc.Bacc("TRN2", debug=False, num_devices=N_CORES)

    ftb = nc.dram_tensor("ftb", [P, KA, BPC, P], f16, kind="ExternalInput").ap()
    wcp = nc.dram_tensor("wcp", [P, PAIRS, 2, C], f16, kind="ExternalInput").ap()
    bcb = nc.dram_tensor("bcb", [P, C], f32, kind="ExternalInput").ap()
    # Output in f16: halves the dominant HBM store traffic (12 MiB/core
    # instead of 24); the host upcasts to f32 during unshard. Rounding
    # adds ~2.4e-4 relative error against a 2e-2 budget.
    out = nc.dram_tensor("out", [BPC, N, C], f16, kind="ExternalOutput").ap()

    with TileContext(nc) as tc:
        with (
            tc.tile_pool(name="consts", bufs=1) as consts,
            tc.tile_pool(name="weights", bufs=1) as weights,
            tc.tile_pool(name="repl", bufs=2) as replp,
            tc.tile_pool(name="ps_b", bufs=1, space="PSUM") as ps_b,
            tc.tile_pool(name="ps_warm", bufs=1, space="PSUM") as ps_warm,
        ):
            # Loads: one whole-tile DMA each (a single completion
            # semaphore per tile posts ~1.5 us after the data lands;
            # partition-split halves doubled that wait), interleaved
            # across the two HWDGE rings by need-time: ft + chunk-pair 0
            # gate the first matmul pass, chunk-pair 1 the fifth, the
            # broadcast bias only the PSUM->SBUF adds.
            ft_sb = consts.tile([P, KA, BPC, P], f16)
            wc_sb = weights.tile([P, PAIRS, 2, C], f16)
            bc_sb = consts.tile([P, C], f32)
            nc.sync.dma_start(out=ft_sb, in_=ftb)
            nc.scalar.dma_start(out=wc_sb[:, 0], in_=wcp[:, 0])
            nc.sync.dma_start(out=wc_sb[:, 1], in_=wcp[:, 1])
            nc.scalar.dma_start(out=bc_sb, in_=bcb)

            # Short PE warm-up on zeroed f16 scratch while loads land.
            dum_l = consts.tile([P, P], f16)
            nc.vector.memset(dum_l, 0.0)
            dum_r = consts.tile([P, 512], f16)
            nc.vector.memset(dum_r, 0.0)
            ps_w = ps_warm.tile([P, 512], f32)
            for _ in range(2):
                nc.tensor.matmul(ps_w, dum_l, dum_r, start=True, stop=True)

            # r4[p, j] = sum_k ftb[k, *, b, p] Wc[j, k] + bc[j]  (same for
            # every p). Two PSUM-bank halves (N=384) x 4 k-chunks per
            # batch; batch 0 fully first so its stores start earliest.
            NS1 = C // 2  # 384
            outv = out.rearrange("b (t p q) c -> b t p (q c)", p=P, q=K_REP)
            engines = [nc.sync, nc.scalar]
            di = 0
            for b in range(BPC):
                pss = [
                    ps_b.tile([P, NS1], f32, name=f"ps_b{b}h{h}")
                    for h in range(2)
                ]
                for a in range(KA):
                    for h in range(2):
                        nc.tensor.matmul(
                            pss[h],
                            ft_sb[:, a, b, :],
                            wc_sb[:, a // 2, a % 2, h * NS1 : (h + 1) * NS1],
                            start=(a == 0),
                            stop=(a == KA - 1),
                        )
                r4 = replp.tile([P, K_REP, C], f16)
                for h in range(2):
                    sl = slice(h * NS1, (h + 1) * NS1)
                    nc.vector.tensor_add(r4[:, 0, sl], pss[h], bc_sb[:, sl])
                # Replication copy on ACT: keeps it off DVE, whose
                # scheduler otherwise slots b=1's bias-add ahead of it.
                nc.scalar.copy(r4[:, 1, :], r4[:, 0, :])
                r4_flat = r4.rearrange("p r c -> p (r c)")
                for t in range(T):
                    engines[di % 2].dma_start(out=outv[b, t], in_=r4_flat)
                    di += 1

    nc.compile()
    return nc


def _get_nc():
    if "nc" not in _CACHE:
        _CACHE["nc"] = _build()
    return _CACHE["nc"]


def _install_ntff_hook():
    """Provide antenv.axon_hooks if the image lacks it (profiling only)."""
    import sys
    import types

    try:
        from antenv.axon_hooks import get_axon_ntff_profile_hook  # noqa: F401

        return
    except ImportError:
        pass
    try:
        import antenv
        from trn_agent_boot.trn_boot import _ntff_profile_via_ctypes

        hook = _ntff_profile_via_ctypes("/opt/axon/libaxon_pjrt.so")
        mod = types.ModuleType("antenv.axon_hooks")
        mod.get_axon_ntff_profile_hook = lambda: hook
        mod.set_axon_ntff_profile_hook = lambda h: None
        sys.modules["antenv.axon_hooks"] = mod
        antenv.axon_hooks = mod
    except Exception as e:  # pragma: no cover - profiling is best-effort
        print(f"ntff hook install failed ({e}); tracing disabled", file=sys.stderr)


def _run(inputs, trace=False):
    from concourse import bass_utils

    if trace:
        _install_ntff_hook()
        # Zero-egress container: skip the artifact upload, keep files local.
        bass_utils.upload_artifacts = lambda tmpdir: tmpdir

    nc = _get_nc()
    ft = np.asarray(inputs["freq_token"], np.float32)
    # Fold the two linear layers (weight preprocessing, float64 for
    # accuracy): out_row = ft @ (Wo @ Wv).T + (Wo @ bv + bo).
    Wv64 = np.asarray(inputs["Wv"], np.float64)
    Wo64 = np.asarray(inputs["Wo"], np.float64)
    bv64 = np.asarray(inputs["bv"], np.float64)
    bo64 = np.asarray(inputs["bo"], np.float64)
    WcT = (Wv64.T @ Wo64.T).astype(np.float16)  # [CFD, C]
    bc = (Wo64 @ bv64 + bo64).astype(np.float32)  # [C]

    # wcp[p, q, j, c] = WcT[(2q+j)*128 + p, c]: each partition's rows for
    # a chunk-pair are contiguous -> 3 KiB load descriptors.
    wcp = np.ascontiguousarray(
        WcT.reshape(PAIRS, 2, P, C).transpose(2, 0, 1, 3)
    )
    bcb = np.ascontiguousarray(np.broadcast_to(bc, (P, C)))

    in_maps = []
    for i in range(N_CORES):
        ft_loc = ft[BPC * i : BPC * (i + 1)]  # [BPC, CFD]
        # ftb[k, a, b, m] = ft_loc[b, a*128 + k] for every m (stationary
        # operand replicated across the 128 M-columns = broadcast form).
        ftb = np.ascontiguousarray(
            np.broadcast_to(
                ft_loc.T.reshape(KA, P, BPC).transpose(1, 0, 2)[:, :, :, None],
                (P, KA, BPC, P),
            ).astype(np.float16)
        )
        in_maps.append({"ftb": ftb, "wcp": wcp, "bcb": bcb})
    res = bass_utils.run_bass_kernel_spmd(
        nc, in_maps, core_ids=list(range(N_CORES)), trace=trace
    )
    out = np.concatenate(
        [m["out"].astype(np.float32) for m in res.results], axis=0
    )
    return out, res


def kernel(**inputs):
    out, _ = _run(inputs, trace=False)
    return out
